# revision 14
# baseline (speedup 1.0000x reference)
"""GAT layer (gnn_message_passing) on 8 Trainium2 NeuronCores.

Strategy (dst-partitioned, replicated projection table, async SWDGE gathers):
  * Nodes padded to NPAD=50176; core p owns dst nodes [p*6272, (p+1)*6272)
    = 49 blocks of 128.
  * Every core computes the full projected table xp = x @ W.T into its DRAM
    as bf16, feature-permuted head-last (j = c*4+h) and pre-scaled by
    att_src (folded into the projection weights).  The projection rhs has 4
    extra columns (W.T @ att_dst fold) so the same matmuls also produce
    a_dst per node; those are stored to a second DRAM table adst_loc with
    256B rows (a_dst[h] in cols 0:4).
  * Edges (+self loops) are bucketed per (core, dst-block) and split into two
    classes by src parity; xp gather index = src//2 (superrow of 1024B = 2
    rows) so indices fit int16.  Each (block,class) cell is padded to
    SUBT*128 (pad index 0, pad dst-slot 200 -> one-hot all zero).
  * Per block: three async dma_gathers (prepare_only + trigger_dma on 4
    rotating SWDGE queues): xp rows for class 0 / class 1 (512B each), and
    per-edge a_dst rows from adst_loc indexed by local dst id (256B each).
    Descriptor generation (~1us) runs on GpSimd; transfers run on the DMA
    rings and overlap with compute.
  * Per block (both classes merged): one-hot oh[edge_p, t, dst] via is_equal
    vs iota; a_src per edge via 2x-mode tree adds + head-wise reduce of the
    pre-scaled gathered rows; ev = a_src + a_dst; w = exp(lrelu(ev)) on the
    scalar engine (parametric relu + exp, same activation table set);
    msg = w * xp_rows; aggregate sum_e w*xp and the denominator with one-hot
    matmuls into a per-block [128, 260] PSUM accumulator.
  * Finalize per block: normalize, undo att_src pre-scale, transpose, fused
    BN+bias affine + ReLU, final linear -> [6272, 64] per core; host
    reassembles [50000, 64] float32.
"""

import numpy as np
import ml_dtypes

BF16 = ml_dtypes.bfloat16

# ---- problem constants ----
N, E, F, H, C = 50000, 800000, 256, 4, 64
NEG_SLOPE = 0.2
BN_EPS = 1e-5
NCORES = 8
BLK = 128
NB = 49                 # dst blocks per core
OWN = NB * BLK          # 6272 dsts per core
NPAD = NCORES * OWN     # 50176
NT = NPAD // 128        # 392 projection tiles
NSUP = NPAD // 2        # 25088 superrows (int16-addressable)

# feature permutation: new index j = c*4 + h  <->  old index h*64 + c
_OLD_OF_NEW = (np.arange(F) % H) * C + (np.arange(F) // H)

LAST_EXEC_NS = None
LAST_RESULTS = None


def _wrap_idx(a):
    """[NCORES, TOT] int -> [NCORES, 128, TOT//16] int16 wrapped+replicated."""
    n = a.shape[1]
    g = a.astype(np.int16).reshape(NCORES, n // 16, 16)
    g = np.ascontiguousarray(g.transpose(0, 2, 1))
    return np.tile(g, (1, 8, 1))


def _prep_edges(edge_index):
    src = np.asarray(edge_index[0], dtype=np.int64)
    dst = np.asarray(edge_index[1], dtype=np.int64)
    src = np.concatenate([src, np.arange(N, dtype=np.int64)])
    dst = np.concatenate([dst, np.arange(N, dtype=np.int64)])

    core = dst // OWN
    dst_local = dst - core * OWN
    block = dst_local // BLK
    dst_slot = (dst_local % BLK).astype(np.float32)
    cls = (src % 2).astype(np.int64)
    gidx = (src // 2).astype(np.int64)                     # superrow index

    ncell_per_core = NB * 2
    cell = core * ncell_per_core + block * 2 + cls
    ncells = NCORES * ncell_per_core
    counts = np.bincount(cell, minlength=ncells).reshape(NCORES, ncell_per_core)
    # per-(block,cls) count (max over cores), 16-aligned; compute shapes use
    # the 128-aligned subtile count
    nie_list = [int(np.ceil(counts[:, ci].max() / 16)) * 16
                for ci in range(ncell_per_core)]
    subt_list = [(n + 127) // 128 for n in nie_list]
    ni_list = [s * 128 for s in subt_list]
    offs = np.zeros(ncell_per_core + 1, dtype=np.int64)
    np.cumsum(ni_list, out=offs[1:])
    TOT = int(offs[-1])

    order = np.argsort(cell, kind="stable")
    sorted_cell = cell[order]
    cell_starts = np.zeros(ncells + 1, dtype=np.int64)
    np.cumsum(counts.reshape(-1), out=cell_starts[1:])
    rank = np.arange(len(order)) - cell_starts[sorted_cell]
    ci_of = sorted_cell % ncell_per_core
    core_of = sorted_cell // ncell_per_core
    flat_pos = core_of * TOT + offs[ci_of] + rank

    gidx_pad = np.zeros(NCORES * TOT, dtype=np.int64)
    gidx_pad[flat_pos] = gidx[order]
    dloc_pad = np.zeros(NCORES * TOT, dtype=np.int64)
    dloc_pad[flat_pos] = dst_local[order]
    dstm_pad = np.full(NCORES * TOT, 200.0, dtype=np.float32)
    dstm_pad[flat_pos] = dst_slot[order]

    idx_all = _wrap_idx(gidx_pad.reshape(NCORES, TOT))     # [8, 128, TOT//16]
    dloc_all = _wrap_idx(dloc_pad.reshape(NCORES, TOT))    # [8, 128, TOT//16]

    # per-core own adst pair-row ids (3136/core, padded to 3200), int16-safe
    NOP = OWN // 2
    ownp = np.zeros((NCORES, 3200), dtype=np.int64)
    for p in range(NCORES):
        ownp[p, :NOP] = p * NOP + np.arange(NOP)
    ownp_all = _wrap_idx(ownp)                             # [8, 128, 200]

    # dst-slot stream in gather layout (edge i at [i%128, i//128]); ragged
    # per-chunk [S_ci * 128] -> [128, S_ci] slices concatenated along free
    d3 = dstm_pad.reshape(NCORES, TOT)
    dst_w = np.empty((NCORES, 128, TOT // 128), dtype=BF16)
    for ci in range(ncell_per_core):
        seg = d3[:, offs[ci]:offs[ci + 1]].reshape(NCORES, subt_list[ci], 128)
        dst_w[:, :, offs[ci] // 128:offs[ci + 1] // 128] = \
            seg.transpose(0, 2, 1).astype(BF16)

    return idx_all, dloc_all, ownp_all, dst_w, (subt_list, nie_list)


def _prep_params(x, W, att_src, att_dst, gat_bias, bn_gamma, bn_beta,
                 bn_mean, bn_var, lin_W, lin_b):
    f32 = np.float32
    W = np.asarray(W, f32)
    att_src_f = np.asarray(att_src, f32).reshape(H * C)      # index h*64+c
    att_dst_f = np.asarray(att_dst, f32)                     # [H, C]

    wt = W.T                                                 # [in, out]
    wt_perm = wt[:, _OLD_OF_NEW] * att_src_f[_OLD_OF_NEW][None, :]
    # a_dst fold: wa[k, h] = sum_c wt[k, h*64+c] * att_dst[h, c]  (UNSCALED)
    wa = np.stack([wt[:, h * C:(h + 1) * C] @ att_dst_f[h] for h in range(H)],
                  axis=1)                                    # [256, 4]
    wt_full = np.concatenate([wt_perm, wa], axis=1)          # [256, 260]
    wt_ext = np.ascontiguousarray(wt_full.reshape(2, 128, 260)).astype(BF16)

    xT = np.zeros((F, NPAD), dtype=f32)
    xT[:, :N] = np.asarray(x, f32).T
    # [NT, 128 partitions, 2 k-chunks, 128 nodes]: partition-major so each
    # tile loads as one contiguous 64KB DMA with 512B per partition
    xT_t = np.ascontiguousarray(
        xT.reshape(2, 128, NT, 128).transpose(2, 1, 0, 3)).astype(BF16)

    att_inv = (1.0 / att_src_f[_OLD_OF_NEW]).astype(f32)
    att_inv_rep = np.tile(att_inv[None, :], (128, 1))

    bnscale = np.asarray(bn_gamma, f32) / np.sqrt(np.asarray(bn_var, f32) + BN_EPS)
    bnshift = ((np.asarray(gat_bias, f32) - np.asarray(bn_mean, f32)) * bnscale
               + np.asarray(bn_beta, f32))
    bnsc = np.ascontiguousarray(bnscale[_OLD_OF_NEW].reshape(2, 128).T)
    bnsh = np.ascontiguousarray(bnshift[_OLD_OF_NEW].reshape(2, 128).T)

    linw = np.asarray(lin_W, f32).T[_OLD_OF_NEW, :]
    linw_t = np.ascontiguousarray(linw.reshape(2, 128, 64)).astype(BF16)
    linb_rep = np.tile(np.asarray(lin_b, f32)[None, :], (128, 1))

    iota_row = np.tile(np.arange(128, dtype=np.float32)[None, :],
                       (128, 1)).astype(BF16)
    ident_f32 = np.eye(128, dtype=np.float32)

    return dict(xT_t=xT_t, wt_ext=wt_ext, att_inv=att_inv_rep.astype(f32),
                bnsc=bnsc.astype(f32), bnsh=bnsh.astype(f32), linw=linw_t,
                linb=linb_rep.astype(f32), iota=iota_row, ident_f32=ident_f32)


def _build(subt_cfg):
    import concourse.bacc as bacc
    import concourse.mybir as mybir
    import concourse.tile as tile

    dt = mybir.dt
    subt_list, nie_list = subt_cfg
    ni_list = [s * 128 for s in subt_list]
    offs = [0]
    for n in ni_list:
        offs.append(offs[-1] + n)
    TOT = offs[-1]
    SMAX2 = max(subt_list[2 * b] + subt_list[2 * b + 1] for b in range(NB))

    nc = bacc.Bacc("TRN2", target_bir_lowering=False, debug=False,
                   enable_asserts=False, num_devices=NCORES)

    xT_in = nc.dram_tensor("xT_t", [NT, 128, 2, 128], dt.bfloat16, kind="ExternalInput")
    wt_in = nc.dram_tensor("wt_ext", [2, 128, 260], dt.bfloat16, kind="ExternalInput")
    attinv_in = nc.dram_tensor("att_inv", [128, 256], dt.float32, kind="ExternalInput")
    bnsc_in = nc.dram_tensor("bnsc", [128, 2], dt.float32, kind="ExternalInput")
    bnsh_in = nc.dram_tensor("bnsh", [128, 2], dt.float32, kind="ExternalInput")
    linw_in = nc.dram_tensor("linw", [2, 128, 64], dt.bfloat16, kind="ExternalInput")
    linb_in = nc.dram_tensor("linb", [128, 64], dt.float32, kind="ExternalInput")
    iota_in = nc.dram_tensor("iota", [128, 128], dt.bfloat16, kind="ExternalInput")
    identf_in = nc.dram_tensor("ident_f32", [128, 128], dt.float32, kind="ExternalInput")
    idx_in = nc.dram_tensor("idx", [128, TOT // 16], dt.int16, kind="ExternalInput")
    dloc_in = nc.dram_tensor("dloc", [128, TOT // 16], dt.int16, kind="ExternalInput")
    ownp_in = nc.dram_tensor("ownp", [128, 200], dt.int16, kind="ExternalInput")
    dstm_in = nc.dram_tensor("dstm", [128, TOT // 128], dt.bfloat16, kind="ExternalInput")
    out_dram = nc.dram_tensor("out", [OWN, 64], dt.float32, kind="ExternalOutput")

    with tile.TileContext(nc) as tc:
        gsems = [nc.alloc_semaphore(f"gq{i}") for i in range(8)]
        gsem = gsems[0]
        with (
            tc.tile_pool(name="dram", bufs=1, space="DRAM") as dramp,
            tc.tile_pool(name="const", bufs=1) as constp,
        ):
            xp_table = dramp.tile([NPAD, 256], dt.bfloat16)
            adst_loc = dramp.tile([NPAD, 128], dt.bfloat16)
            adst_own = dramp.tile([6400, 128], dt.bfloat16)
            # superrow views: [25088, 512] -> even/odd 256-col halves
            sup = xp_table[:].rearrange("(s two) f -> s (two f)", two=2)

            wt_sb = constp.tile([128, 2, 260], dt.bfloat16)
            for k in range(2):
                nc.sync.dma_start(out=wt_sb[:, k, :], in_=wt_in[k])

            # ---- phase A: projection (+ per-node a_dst cols 256:260) ----
            with (
                tc.tile_pool(name="proj_sb", bufs=3) as psb,
                tc.tile_pool(name="proj_acc", bufs=2) as accp,
                tc.tile_pool(name="proj_ps", bufs=3, space="PSUM") as pps,
            ):
                acc = accp.tile([128, 8, 4], dt.bfloat16, tag="acc")
                for ntile in range(NT):
                    xt = psb.tile([128, 2, 128], dt.bfloat16, tag="xt")
                    nc.sync.dma_start(out=xt[:], in_=xT_in[ntile])
                    ps = pps.tile([128, 260], dt.float32, space="PSUM")
                    nc.tensor.matmul(out=ps[:], lhsT=xt[:, 0, :],
                                     rhs=wt_sb[:, 0, :], start=True, stop=False)
                    nc.tensor.matmul(out=ps[:], lhsT=xt[:, 1, :],
                                     rhs=wt_sb[:, 1, :], start=False, stop=True)
                    xp_sb = psb.tile([128, 256], dt.bfloat16, tag="xp")
                    nc.vector.tensor_copy(out=xp_sb[:], in_=ps[:, 0:256])
                    nc.vector.tensor_copy(out=acc[:, ntile % 8, :],
                                          in_=ps[:, 256:260])
                    nc.scalar.dma_start(
                        out=xp_table[ntile * 128:(ntile + 1) * 128, :],
                        in_=xp_sb[:])
                    if ntile % 8 == 7:
                        g0 = ntile - 7
                        nc.sync.dma_start(
                            out=adst_loc[g0 * 128:(g0 + 8) * 128, 0:4]
                                .rearrange("(g q) h -> q g h", q=128),
                            in_=acc[:, 0:8, :])
                        if ntile != NT - 1:
                            acc = accp.tile([128, 8, 4], dt.bfloat16, tag="acc")

            # ---- phase B consts ----
            att_inv_sb = constp.tile([128, 256], dt.float32)
            nc.sync.dma_start(out=att_inv_sb[:], in_=attinv_in[:])
            bnsc_sb = constp.tile([128, 2], dt.float32)
            nc.sync.dma_start(out=bnsc_sb[:], in_=bnsc_in[:])
            bnsh_sb = constp.tile([128, 2], dt.float32)
            nc.sync.dma_start(out=bnsh_sb[:], in_=bnsh_in[:])
            linw_sb = constp.tile([128, 2, 64], dt.bfloat16)
            for k in range(2):
                nc.sync.dma_start(out=linw_sb[:, k, :], in_=linw_in[k])
            linb_sb = constp.tile([128, 64], dt.float32)
            nc.sync.dma_start(out=linb_sb[:], in_=linb_in[:])
            iota_sb = constp.tile([128, 128], dt.bfloat16)
            nc.sync.dma_start(out=iota_sb[:], in_=iota_in[:])
            identf_sb = constp.tile([128, 128], dt.float32)
            nc.sync.dma_start(out=identf_sb[:], in_=identf_in[:])
            idx_sb = constp.tile([128, TOT // 16], dt.int16)
            nc.sync.dma_start(out=idx_sb[:], in_=idx_in[:])
            dloc_sb = constp.tile([128, TOT // 16], dt.int16)
            nc.sync.dma_start(out=dloc_sb[:], in_=dloc_in[:])
            dstm_sb = constp.tile([128, TOT // 128], dt.bfloat16)
            nc.sync.dma_start(out=dstm_sb[:], in_=dstm_in[:])
            ownp_sb = constp.tile([128, 200], dt.int16)
            nc.sync.dma_start(out=ownp_sb[:], in_=ownp_in[:])

            # ---- phase B: per-block pipeline ----
            with (
                tc.tile_pool(name="gsb", bufs=3) as gsb,
                tc.tile_pool(name="ohsb", bufs=3) as ohsb,
                tc.tile_pool(name="msb", bufs=2) as msb,
                tc.tile_pool(name="fsb", bufs=2) as fsb,
                tc.tile_pool(name="aggps", bufs=3, space="PSUM") as aggps,
                tc.tile_pool(name="tps", bufs=2, space="PSUM") as tps,
                tc.tile_pool(name="finps", bufs=1, space="PSUM") as finps,
            ):
                # own a_dst rows: gather this core's 3136 pair-rows (512B)
                # from the global table, unpack to per-node 256B rows
                pairs = adst_loc[:].rearrange("(j two) e -> j (two e)", two=2)
                xo2 = gsb.tile([128, 25, 256], dt.bfloat16, tag="xo2")
                for g0 in range(0, 25, 8):
                    gs = min(8, 25 - g0)
                    nc.gpsimd.dma_gather(
                        out_ap=xo2[:, g0:g0 + gs, :], in_ap=pairs,
                        idxs_ap=ownp_sb[:, g0 * 8:(g0 + gs) * 8],
                        num_idxs=gs * 128, num_idxs_reg=gs * 128,
                        elem_size=256, elem_step=256,
                        prepare_only=True, sem=gsem)
                    nc.gpsimd.trigger_dma(count=None)
                own_v = adst_own[:].rearrange("(t q two) e -> q t two e",
                                              q=128, two=2)
                nc.sync.dma_start(out=own_v[:, :, 0, 0:4],
                                  in_=xo2[:, :, 0:4])
                nc.sync.dma_start(out=own_v[:, :, 1, 0:4],
                                  in_=xo2[:, :, 128:132])
                for b in range(NB):
                    S0 = subt_list[2 * b]
                    S1 = subt_list[2 * b + 1]
                    ST = S0 + S1
                    oW0 = offs[2 * b] // 16
                    oW1 = offs[2 * b + 1] // 16
                    oS = offs[2 * b] // 128

                    xg = gsb.tile([128, SMAX2, 256], dt.bfloat16, tag="xg")
                    adg = gsb.tile([128, SMAX2, 128], dt.bfloat16, tag="adg")
                    for (lo, S, oW, col0) in ((0, S0, oW0, 0),
                                              (S0, S1, oW1, 256)):
                        for g0 in range(0, S, 8):
                            gs = min(8, S - g0)
                            nc.gpsimd.dma_gather(
                                out_ap=xg[:, lo + g0:lo + g0 + gs, :],
                                in_ap=sup[:, col0:col0 + 256],
                                idxs_ap=idx_sb[:, oW + g0 * 8:
                                               oW + (g0 + gs) * 8],
                                num_idxs=gs * 128, num_idxs_reg=gs * 128,
                                elem_size=256, elem_step=512,
                                prepare_only=True, sem=gsem)
                            nc.gpsimd.trigger_dma(count=None)
                    for g0 in range(0, ST, 8):
                        gs = min(8, ST - g0)
                        nc.gpsimd.dma_gather(
                            out_ap=adg[:, g0:g0 + gs, :], in_ap=adst_own[:],
                            idxs_ap=dloc_sb[:, oW0 + g0 * 8:
                                            oW0 + (g0 + gs) * 8],
                            num_idxs=gs * 128, num_idxs_reg=gs * 128,
                            elem_size=128, elem_step=128,
                            prepare_only=True, sem=gsem)
                        nc.gpsimd.trigger_dma(count=None)

                    # one-hot [edge_p, t, dst]
                    oh = ohsb.tile([128, SMAX2, 128], dt.bfloat16, tag="oh")
                    nc.vector.tensor_tensor(
                        out=oh[:, 0:ST, :],
                        in0=dstm_sb[:, oS:oS + ST, None].to_broadcast(
                            [128, ST, 128]),
                        in1=iota_sb[:, None, :].to_broadcast([128, ST, 128]),
                        op=mybir.AluOpType.is_equal)
                    # a_src: head-wise row sums via 2x-mode tree adds
                    xg4 = xg[:, 0:ST, :].rearrange("p t (c h) -> p t c h", h=H)
                    tr1 = msb.tile([128, SMAX2, 32, 4], dt.bfloat16, tag="tr1")
                    nc.vector.tensor_tensor(
                        out=tr1[:, 0:ST, :, :], in0=xg4[:, :, 0:32, :],
                        in1=xg4[:, :, 32:64, :], op=mybir.AluOpType.add)
                    tr2 = msb.tile([128, SMAX2, 16, 4], dt.bfloat16, tag="tr2")
                    nc.vector.tensor_tensor(
                        out=tr2[:, 0:ST, :, :], in0=tr1[:, 0:ST, 0:16, :],
                        in1=tr1[:, 0:ST, 16:32, :], op=mybir.AluOpType.add)
                    tr3 = msb.tile([128, SMAX2, 8, 4], dt.bfloat16, tag="tr3")
                    nc.vector.tensor_tensor(
                        out=tr3[:, 0:ST, :, :], in0=tr2[:, 0:ST, 0:8, :],
                        in1=tr2[:, 0:ST, 8:16, :], op=mybir.AluOpType.add)
                    asrc = msb.tile([128, SMAX2, 4], dt.float32, tag="asrc")
                    nc.vector.reduce_sum(
                        out=asrc[:, 0:ST, :],
                        in_=tr3[:, 0:ST, :, :].rearrange("p t c h -> p t h c"),
                        axis=mybir.AxisListType.X)
                    # ev = a_src + a_dst ; w = exp(lrelu(ev)) on scalar engine
                    ev = msb.tile([128, SMAX2, 4], dt.float32, tag="ev")
                    nc.vector.tensor_tensor(out=ev[:, 0:ST, :],
                                            in0=asrc[:, 0:ST, :],
                                            in1=adg[:, 0:ST, 0:4],
                                            op=mybir.AluOpType.add)
                    # w = exp(lrelu(ev)) = max(exp(ev), exp(0.2*ev))
                    msg = msb.tile([128, SMAX2, 260], dt.bfloat16, tag="msg")
                    nc.scalar.activation(msg[:, 0:ST, 256:260], ev[:, 0:ST, :],
                                         mybir.ActivationFunctionType.Exp)
                    e2 = msb.tile([128, SMAX2, 4], dt.bfloat16, tag="e2")
                    nc.scalar.activation(e2[:, 0:ST, :], ev[:, 0:ST, :],
                                         mybir.ActivationFunctionType.Exp,
                                         scale=NEG_SLOPE)
                    nc.vector.tensor_tensor(out=msg[:, 0:ST, 256:260],
                                            in0=msg[:, 0:ST, 256:260],
                                            in1=e2[:, 0:ST, :],
                                            op=mybir.AluOpType.max)
                    nc.vector.tensor_tensor(
                        out=msg[:, 0:ST, 0:256].rearrange(
                            "p t (c h) -> p t c h", h=H),
                        in0=xg4[:],
                        in1=msg[:, 0:ST, 256:260][:, :, None, :]
                            .to_broadcast([128, ST, C, H]),
                        op=mybir.AluOpType.mult)
                    agg = aggps.tile([128, 260], dt.float32, space="PSUM")
                    for t in range(ST):
                        nc.tensor.matmul(
                            out=agg[:], lhsT=oh[:, t, :], rhs=msg[:, t, :],
                            start=(t == 0), stop=(t == ST - 1))
                    # ---- finalize ----
                    den = fsb.tile([128, 4], dt.float32, tag="den")
                    nc.vector.tensor_scalar_add(den[:], agg[:, 256:260], 1e-30)
                    rec = fsb.tile([128, 4], dt.float32, tag="rec")
                    nc.vector.reciprocal(rec[:], den[:])
                    gat_u = fsb.tile([128, 256], dt.float32, tag="gat_u")
                    nc.vector.tensor_tensor(out=gat_u[:], in0=agg[:, 0:256],
                                            in1=att_inv_sb[:],
                                            op=mybir.AluOpType.mult)
                    gat = fsb.tile([128, 256], dt.float32, tag="gat")
                    nc.vector.tensor_tensor(
                        out=gat[:].rearrange("p (c h) -> p c h", h=H),
                        in0=gat_u[:].rearrange("p (c h) -> p c h", h=H),
                        in1=rec[:, None, :].to_broadcast([128, C, H]),
                        op=mybir.AluOpType.mult)
                    fps = finps.tile([128, 64], dt.float32, space="PSUM")
                    gt = fsb.tile([128, 2, 128], dt.bfloat16, tag="gt")
                    for k in range(2):
                        pst = tps.tile([128, 128], dt.float32, space="PSUM",
                                       tag="pst")
                        nc.tensor.transpose(out=pst[:],
                                            in_=gat[:, k * 128:(k + 1) * 128],
                                            identity=identf_sb[:])
                        nc.scalar.activation(gt[:, k, :], pst[:],
                                             mybir.ActivationFunctionType.Relu,
                                             bias=bnsh_sb[:, k:k + 1],
                                             scale=bnsc_sb[:, k:k + 1])
                        nc.tensor.matmul(out=fps[:], lhsT=gt[:, k, :],
                                         rhs=linw_sb[:, k, :],
                                         start=(k == 0), stop=(k == 1))
                    ob = fsb.tile([128, 64], dt.float32, tag="ob")
                    nc.vector.tensor_tensor(out=ob[:], in0=fps[:],
                                            in1=linb_sb[:],
                                            op=mybir.AluOpType.add)
                    nc.sync.dma_start(
                        out=out_dram[b * 128:(b + 1) * 128, :], in_=ob[:])

    # Tile gates consumers of prepare_only gather outputs on per-lane DMASW
    # sems that it bumps EAGERLY (IncSwdgeSem at trigger time, before the DMA
    # lands) -- racy on HW and in sim.  The true completion signal is the
    # descriptor sem (OnUpdate[0], +16 per prep at DMA completion).  Mirror
    # Tile's 8-lane scheme on our own sems: prep j (Pool scheduled order)
    # gets descriptor sem gq[j%8]; every consumer DMASW{k} wait is repointed
    # to gq[k] with its threshold unchanged.
    import re as _re
    sem_ids = {}
    for _sid, _names in dict(nc.m.ant_sem_names).items():
        for _nm in _names:
            sem_ids[_nm] = int(_sid)
    gq_ids = [sem_ids[f"gq{k}"] for k in range(8)]
    ins_all = [i for blk in nc.m.functions[0].blocks for i in blk.instructions]
    j = 0
    for x in ins_all:
        if type(x).__name__ == "InstDMAGatherAnt":
            u0 = x.sync_info.on_update[0]
            u0.id = gq_ids[j % 8]
            u0.ant_name = f"gq{j % 8}"
            j += 1
    for x in ins_all:
        si = x.sync_info
        if not si:
            continue
        for w in si.on_wait:
            mm = _re.match(r"DMASW(\d+)_", w.ant_name)
            if mm:
                k = int(mm.group(1))
                assert w.wait_value % 16 == 0
                w.id = gq_ids[k]
                w.ant_name = f"gq{k}"

    nc.compile()
    return nc


def _install_ntff_shim():
    """Install the axon NTFF profiling hook (missing antenv.axon_hooks shim)."""
    import sys, types
    if "antenv.axon_hooks" in sys.modules:
        return
    m = types.ModuleType("antenv.axon_hooks")
    _h = [None]
    m.set_axon_ntff_profile_hook = lambda h: _h.__setitem__(0, h)
    m.get_axon_ntff_profile_hook = lambda: _h[0]
    sys.modules["antenv.axon_hooks"] = m
    import antenv
    antenv.axon_hooks = m
    from trn_agent_boot.trn_boot import _ntff_profile_via_ctypes
    hook = _ntff_profile_via_ctypes("/opt/axon/libaxon_pjrt.so")
    if hook is not None:
        m.set_axon_ntff_profile_hook(hook)


def kernel(**inputs):
    global LAST_EXEC_NS, LAST_RESULTS
    import os
    from concourse import bass_utils

    trace = os.environ.get("KERNEL_TRACE") == "1"
    if trace:
        try:
            _install_ntff_shim()
            bass_utils.upload_artifacts = lambda tmpdir: "(upload skipped)"
        except Exception as e:
            print("ntff shim failed:", e)
            trace = False

    idx_all, dloc_all, ownp_all, dst_all, subt_cfg = _prep_edges(
        np.asarray(inputs["edge_index"]))
    params = _prep_params(
        inputs["x"], inputs["W"], inputs["att_src"], inputs["att_dst"],
        inputs["gat_bias"], inputs["bn_gamma"], inputs["bn_beta"],
        inputs["bn_mean"], inputs["bn_var"], inputs["lin_W"], inputs["lin_b"])

    nc = _build(subt_cfg)

    shared = dict(
        xT_t=params["xT_t"], wt_ext=params["wt_ext"], att_inv=params["att_inv"],
        bnsc=params["bnsc"], bnsh=params["bnsh"], linw=params["linw"],
        linb=params["linb"], iota=params["iota"],
        ident_f32=params["ident_f32"])
    in_maps = []
    for p in range(NCORES):
        m = dict(shared)
        m["idx"] = np.ascontiguousarray(idx_all[p])
        m["dloc"] = np.ascontiguousarray(dloc_all[p])
        m["ownp"] = np.ascontiguousarray(ownp_all[p])
        m["dstm"] = np.ascontiguousarray(dst_all[p])
        in_maps.append(m)

    run_kwargs = {}
    if trace:
        run_kwargs = dict(trace=True, tmpdir=os.environ.get(
            "KERNEL_TRACE_DIR", "/tmp/gat_prof"))
        os.makedirs(run_kwargs["tmpdir"], exist_ok=True)
    res = bass_utils.run_bass_kernel_spmd(
        nc, in_maps, core_ids=list(range(NCORES)), **run_kwargs)
    LAST_EXEC_NS = res.exec_time_ns
    LAST_RESULTS = res

    full = np.empty((NPAD, 64), dtype=np.float32)
    for p in range(NCORES):
        full[p * OWN:(p + 1) * OWN] = res.results[p]["out"]
    return full[:N]
